# revision 21
# baseline (speedup 1.0000x reference)
"""Trainium2 Bass kernel for ConcentrationLoss.

Math (per batch element b, fully independent across b):
    g      = grid[b] viewed as (2, 4096)            # channels x pixels
    coord1 = g @ aff[b]                             # (2, 4096), the heavy op
    view coord1 as (2, 64, 64); extract 8x8 windows stride 4 -> 15x15 windows
    loss contribution = sum over windows w of [ sum_{p in w} x_p^2 - (sum_{p in w} x_p)^2 / 64 ]
    final = sum_b contribution_b / (8 * 2 * 225 * 64)

Sharding: batch b -> core b (8 cores). Each core streams its 64MB aff slice
through the TensorEngine (memory-bound), reduces the windowed variance on
device to per-channel partial sums, and the host combines the 8 partial
results into the scalar.

Device pipeline per core:
  - Main matmul: out = lhsT.T @ rhs with lhsT = g^T chunks (128, 2) and
    rhs = aff row-band tiles (128, 512), accumulated into PSUM (2, 4096)
    over the 32 contraction chunks. Operands are float32r: single-pass
    matmul at 1 col/cycle (fp32 would stream aff through the PE twice at
    half rate). fp32r rounds operands to ~12 mantissa bits; the end-to-end
    loss error stays ~1e-6 because the truncation noise averages out over
    the 230K-element mean.
  - As soon as PSUM bank n (512 pixels = 8 image rows) finishes
    accumulating, its post-processing overlaps the remaining stream:
    square (ACT), w-direction window sums of x and x^2 (8 strided adds
    each on DVE / GpSimd) into Y/Ysq (2, 64, 15).
  - After the last bank: h-direction window sums (8 strided adds) give
    S/SSq (2, 15, 15); then sum(SSq) and sum(S^2) reduce to a (2, 2)
    output. Host: loss_b = sum_c [ sumSSq_c - sumS2_c / 64 ].
"""

import numpy as np

B = 8
C = 2
H = W = 64
PIX = H * W  # 4096, contraction dim
WIN = 8
STRIDE = 4
OH = OW = 15
KC = PIX // 128  # 32 contraction chunks of 128
NT = PIX // 512  # 8 psum-bank-wide output chunks (column slabs)
ROWS_PER_BANK = 512 // W  # 8 image rows per psum bank
KPT = 8          # contraction chunks per DMA tile (tile = 128 x KPT x 512 = 2MB)
TPS = KC // KPT  # DMA tiles per slab
AFF_BUFS = 6
USE_F32R = True  # fp32r matmul: 1 cycle/col-pass (fp32 streams aff twice); ~2.8e-4 operand rounding

_CACHE = {}


def _split_multi_waits(nc, limit=1):
    """The walrus build in this toolchain rejects instructions carrying more
    than one sync wait (any template: CTRL, S3_LW, ...). Tile's scheduler
    freely emits multi-wait instructions. Post-process the scheduled BIR:
    hoist excess waits onto one-wait NoOps inserted immediately before the
    instruction on the same engine (sequencer waits are conjunctive and
    blocking, so semantics are identical)."""
    import concourse.mybir as mybir

    n_split = 0
    for f in nc.m.functions:
        for b in f.blocks:
            insts = b.instructions  # live view
            i = 0
            while i < len(insts):
                inst = insts[i]
                si = inst.sync_info
                if si is not None and len(si.on_wait) > limit:
                    waits = list(si.on_wait)
                    extra, keep = waits[:-limit], waits[-limit:]
                    for w in extra:
                        nop = mybir.InstNoOp(name=f"SWS-{n_split}")
                        n_split += 1
                        nop.engine = inst.engine
                        nop.sync_info = mybir.SyncInfo(on_wait=[w], on_update=[])
                        insts.insert(i, nop)
                        i += 1
                    inst.sync_info = mybir.SyncInfo(
                        on_wait=keep, on_update=si.on_update
                    )
                i += 1
    return n_split


def _build_nc():
    import concourse.bass as bass
    import concourse.mybir as mybir
    import concourse.tile as tile

    f32 = mybir.dt.float32
    fmm = mybir.dt.float32r if USE_F32R else f32
    nc = bass.Bass()
    # aff is pre-packed on the host into DMA-tile order [slab, tile, j, p, n]
    # (j = 8KB half-tile): every transfer is one contiguous 2MB block whose
    # descriptors split into balanced 8KB runs (16KB runs overload the slow
    # SDMA engine 15 and it straggles ~15%)
    aff = nc.dram_tensor(
        "aff", [NT, TPS, 2, 128, KPT // 2 * 512], fmm, kind="ExternalInput"
    )
    gt = nc.dram_tensor("gt", [128, 2 * KC], fmm, kind="ExternalInput")
    out = nc.dram_tensor("out", [C, 2], f32, kind="ExternalOutput")

    with tile.TileContext(nc) as tc:
        with (
            tc.tile_pool(name="consts", bufs=1) as consts,
            tc.tile_pool(name="small", bufs=1) as small,
            tc.tile_pool(name="sqp", bufs=2) as sqp,
            tc.tile_pool(name="affp", bufs=AFF_BUFS) as affp,
            tc.tile_pool(name="ps1", bufs=1, space="PSUM") as ps1,
        ):
            # consts go through SWDGE (gpsimd) so they never queue behind the
            # big aff stream on the HWDGE rings
            gt_sb = consts.tile([128, 2 * KC], fmm)
            nc.gpsimd.dma_start(out=gt_sb, in_=gt[:, :])

            y_sb = small.tile([C, H, OW], f32)      # w-windowsums of x
            ysq_sb = small.tile([C, H, OW], f32)    # w-windowsums of x^2
            s_sb = small.tile([C, OH * OW], f32)    # full window sums
            ssq_sb = small.tile([C, OH * OW], f32)  # full window sums of x^2
            s2_sb = small.tile([C, OH * OW], f32)   # S^2
            out_sb = small.tile([C, 2], f32)


            def windowed(ap, row_step, n_rows):
                """4-dim overlapping AP: [part, row, window j, dw] over a
                (C, n_rows*row_step) region; one tensor_reduce(X) gives the
                w-direction window sums in a single instruction."""
                return bass.AP(
                    tensor=ap.tensor,
                    offset=ap.offset,
                    ap=[list(ap.ap[0]), [row_step, n_rows], [STRIDE, OW], [1, WIN]],
                )

            def bank_postprocess(n, bank):
                """w-direction window sums for psum bank n; overlaps stream."""
                sq = sqp.tile([C, 512], f32, tag="sq")
                nc.scalar.square(out=sq, in_=bank)
                yd = y_sb[:, n * ROWS_PER_BANK:(n + 1) * ROWS_PER_BANK, :]
                qd = ysq_sb[:, n * ROWS_PER_BANK:(n + 1) * ROWS_PER_BANK, :]
                nc.vector.reduce_sum(
                    out=yd, in_=windowed(bank, W, ROWS_PER_BANK),
                    axis=mybir.AxisListType.X,
                )
                nc.vector.reduce_sum(
                    out=qd, in_=windowed(sq[:, :], W, ROWS_PER_BANK),
                    axis=mybir.AxisListType.X,
                )

            # h-direction window sums, incremental: S[c, i, j] = sum_dh
            # Y[c, 4i+dh, j]. Window row i needs Y rows 4i..4i+7; after bank
            # n the rows up to 8n+7 exist, so rows {2n-1, 2n} (and row 0 for
            # n=0) become computable.
            sv = s_sb.rearrange("c (i j) -> c i j", j=OW)
            qv = ssq_sb.rearrange("c (i j) -> c i j", j=OW)

            def h_rows(i0, cnt):
                for src, dst in ((y_sb, sv), (ysq_sb, qv)):
                    ap = src[:, :, :]
                    win = bass.AP(
                        tensor=ap.tensor,
                        offset=ap.offset + i0 * STRIDE * OW,
                        ap=[list(ap.ap[0]), [STRIDE * OW, cnt], [1, OW], [OW, WIN]],
                    )
                    nc.vector.reduce_sum(
                        out=dst[:, i0:i0 + cnt, :], in_=win,
                        axis=mybir.AxisListType.X,
                    )

            # column-slab-major stream: all 32 contraction chunks for one
            # 512-col psum bank, then the next. Banks finish progressively,
            # so the windowed reduction overlaps the stream.
            for s in range(NT):
                # one PSUM tile per bank so the post-stream reads never
                # create WAR hazards against later banks' matmuls
                c1b = ps1.tile([C, 512], f32, tag="bank", bufs=NT, name=f"c1b{s}")
                for t in range(TPS):
                    at = affp.tile([128, 2, KPT // 2, 512], fmm)
                    nc.sync.dma_start(
                        out=at,
                        in_=aff[s, t].rearrange("j p n -> p j n").rearrange(
                            "p j (q n) -> p j q n", n=512
                        ),
                    )
                    for j in range(2):
                        for q in range(KPT // 2):
                            kc = t * KPT + j * (KPT // 2) + q
                            nc.tensor.matmul(
                                c1b,
                                lhsT=gt_sb[:, 2 * kc:2 * kc + 2],
                                rhs=at[:, j, q, :],
                                start=(kc == 0),
                                stop=(kc == KC - 1),
                            )
                bank_postprocess(s, c1b)
                if s == 0:
                    h_rows(0, 1)
                else:
                    h_rows(2 * s - 1, 2)

            nc.scalar.square(out=s2_sb, in_=s_sb)
            nc.vector.reduce_sum(out=out_sb[:, 0:1], in_=ssq_sb, axis=mybir.AxisListType.X)
            nc.vector.reduce_sum(out=out_sb[:, 1:2], in_=s2_sb, axis=mybir.AxisListType.X)
            nc.sync.dma_start(out=out[:, :], in_=out_sb)
    _split_multi_waits(nc)
    return nc


def _gt_host(grid_b):
    # grid_b: (64, 64, 2). g[c, p] = grid_b.reshape(4096, 2)[p, c]
    # gt layout: gt[p, 2*kc + c] = g[c, 128*kc + p]
    gt = np.ascontiguousarray(grid_b, dtype=np.float32).reshape(PIX, C)
    return np.ascontiguousarray(
        gt.reshape(KC, 128, C).transpose(1, 0, 2).reshape(128, 2 * KC)
    )


def run_cores(aff, grid, trace=False):
    """Compile (cached) and run the per-core bass kernel on cores 0..7.

    Returns the BassKernelResults from run_bass_kernel_spmd."""
    from concourse.bass_utils import run_bass_kernel_spmd

    if "nc" not in _CACHE:
        _CACHE["nc"] = _build_nc()
    nc = _CACHE["nc"]

    in_maps = []
    for b in range(B):
        # pack aff into DMA-tile order [slab, tile, j, p, (q n)]: element
        # [(t*KPT + 4j + q)*128 + p, s*512 + n] -> [s, t, j, p, q, n]
        a = np.ascontiguousarray(aff[b], dtype=np.float32)
        a = a.reshape(TPS, 2, KPT // 2, 128, NT, 512).transpose(4, 0, 1, 3, 2, 5)
        a = a.reshape(NT, TPS, 2, 128, KPT // 2 * 512)
        in_maps.append(
            {
                "aff": np.ascontiguousarray(a),
                "gt": _gt_host(grid[b]),
            }
        )
    return run_bass_kernel_spmd(nc, in_maps, core_ids=list(range(B)), trace=trace)


def kernel(aff, grid):
    aff = np.asarray(aff, dtype=np.float32)
    grid = np.asarray(grid, dtype=np.float32)
    res = run_cores(aff, grid)
    total = 0.0
    for b in range(B):
        o = res.results[b]["out"].astype(np.float64)
        total += o[:, 0].sum() - o[:, 1].sum() / (WIN * WIN)
    total /= B * C * OH * OW * WIN * WIN
    return np.asarray(total, dtype=np.float32)


# revision 22
# speedup vs baseline: 1.1459x; 1.1459x over previous
"""Trainium2 Bass kernel for ConcentrationLoss.

Math (per batch element b, fully independent across b):
    g      = grid[b] viewed as (2, 4096)            # channels x pixels
    coord1 = g @ aff[b]                             # (2, 4096), the heavy op
    view coord1 as (2, 64, 64); extract 8x8 windows stride 4 -> 15x15 windows
    loss contribution = sum over windows w of [ sum_{p in w} x_p^2 - (sum_{p in w} x_p)^2 / 64 ]
    final = sum_b contribution_b / (8 * 2 * 225 * 64)

Sharding: batch b -> core b (8 cores). Each core streams its 64MB aff slice
through the TensorEngine (memory-bound), reduces the windowed variance on
device to per-channel partial sums, and the host combines the 8 partial
results into the scalar.

Device pipeline per core:
  - Main matmul: out = lhsT.T @ rhs with lhsT = g^T chunks (128, 2) and
    rhs = aff row-band tiles (128, 512), accumulated into PSUM (2, 4096)
    over the 32 contraction chunks. Operands are float32r: single-pass
    matmul at 1 col/cycle (fp32 would stream aff through the PE twice at
    half rate). fp32r rounds operands to ~12 mantissa bits; the end-to-end
    loss error stays ~1e-6 because the truncation noise averages out over
    the 230K-element mean.
  - As soon as PSUM bank n (512 pixels = 8 image rows) finishes
    accumulating, its post-processing overlaps the remaining stream:
    square (ACT), w-direction window sums of x and x^2 (8 strided adds
    each on DVE / GpSimd) into Y/Ysq (2, 64, 15).
  - After the last bank: h-direction window sums (8 strided adds) give
    S/SSq (2, 15, 15); then sum(SSq) and sum(S^2) reduce to a (2, 2)
    output. Host: loss_b = sum_c [ sumSSq_c - sumS2_c / 64 ].
"""

import numpy as np

B = 8
C = 2
H = W = 64
PIX = H * W  # 4096, contraction dim
WIN = 8
STRIDE = 4
OH = OW = 15
KC = PIX // 128  # 32 contraction chunks of 128
NT = PIX // 512  # 8 psum-bank-wide output chunks (column slabs)
ROWS_PER_BANK = 512 // W  # 8 image rows per psum bank
SLAB_BANKS = 4   # psum banks (column groups) per slab
NSLAB = NT // SLAB_BANKS
KPT = 2          # contraction chunks per DMA tile (tile = 128 x KPT x 2048 = 2MB)
TPS = KC // KPT  # DMA tiles per slab
AFF_BUFS = 6
USE_F32R = True  # fp32r matmul: 1 cycle/col-pass (fp32 streams aff twice); ~2.8e-4 operand rounding

_CACHE = {}


def _split_multi_waits(nc, limit=1):
    """The walrus build in this toolchain rejects instructions carrying more
    than one sync wait (any template: CTRL, S3_LW, ...). Tile's scheduler
    freely emits multi-wait instructions. Post-process the scheduled BIR:
    hoist excess waits onto one-wait NoOps inserted immediately before the
    instruction on the same engine (sequencer waits are conjunctive and
    blocking, so semantics are identical)."""
    import concourse.mybir as mybir

    n_split = 0
    for f in nc.m.functions:
        for b in f.blocks:
            insts = b.instructions  # live view
            i = 0
            while i < len(insts):
                inst = insts[i]
                si = inst.sync_info
                if si is not None and len(si.on_wait) > limit:
                    waits = list(si.on_wait)
                    extra, keep = waits[:-limit], waits[-limit:]
                    for w in extra:
                        nop = mybir.InstNoOp(name=f"SWS-{n_split}")
                        n_split += 1
                        nop.engine = inst.engine
                        nop.sync_info = mybir.SyncInfo(on_wait=[w], on_update=[])
                        insts.insert(i, nop)
                        i += 1
                    inst.sync_info = mybir.SyncInfo(
                        on_wait=keep, on_update=si.on_update
                    )
                i += 1
    return n_split


def _build_nc():
    import concourse.bass as bass
    import concourse.mybir as mybir
    import concourse.tile as tile

    f32 = mybir.dt.float32
    fmm = mybir.dt.float32r if USE_F32R else f32
    nc = bass.Bass()
    # aff is pre-packed on the host into DMA-tile order [slab, tile, j, p, n]
    # (j = 8KB half-tile = one contraction chunk): every transfer is one
    # contiguous 2MB block whose descriptors split into balanced 8KB runs
    aff = nc.dram_tensor(
        "aff", [NSLAB, TPS, KPT, 128, SLAB_BANKS * 512], fmm, kind="ExternalInput"
    )
    gt = nc.dram_tensor("gt", [128, 2 * KC], fmm, kind="ExternalInput")
    out = nc.dram_tensor("out", [C, 2], f32, kind="ExternalOutput")

    with tile.TileContext(nc) as tc:
        with (
            tc.tile_pool(name="consts", bufs=1) as consts,
            tc.tile_pool(name="small", bufs=1) as small,
            tc.tile_pool(name="sqp", bufs=2) as sqp,
            tc.tile_pool(name="affp", bufs=AFF_BUFS) as affp,
            tc.tile_pool(name="ps1", bufs=1, space="PSUM") as ps1,
        ):
            # consts go through SWDGE (gpsimd) so they never queue behind the
            # big aff stream on the HWDGE rings
            gt_sb = consts.tile([128, 2 * KC], fmm)
            nc.gpsimd.dma_start(out=gt_sb, in_=gt[:, :])

            y_sb = small.tile([C, H, OW], f32)      # w-windowsums of x
            ysq_sb = small.tile([C, H, OW], f32)    # w-windowsums of x^2
            s_sb = small.tile([C, OH * OW], f32)    # full window sums
            ssq_sb = small.tile([C, OH * OW], f32)  # full window sums of x^2
            s2_sb = small.tile([C, OH * OW], f32)   # S^2
            out_sb = small.tile([C, 2], f32)


            def windowed(ap, row_step, n_rows):
                """4-dim overlapping AP: [part, row, window j, dw] over a
                (C, n_rows*row_step) region; one tensor_reduce(X) gives the
                w-direction window sums in a single instruction."""
                return bass.AP(
                    tensor=ap.tensor,
                    offset=ap.offset,
                    ap=[list(ap.ap[0]), [row_step, n_rows], [STRIDE, OW], [1, WIN]],
                )

            def bank_postprocess(n, bank):
                """w-direction window sums for psum bank n; overlaps stream."""
                sq = sqp.tile([C, 512], f32, tag="sq")
                nc.scalar.square(out=sq, in_=bank)
                yd = y_sb[:, n * ROWS_PER_BANK:(n + 1) * ROWS_PER_BANK, :]
                qd = ysq_sb[:, n * ROWS_PER_BANK:(n + 1) * ROWS_PER_BANK, :]
                nc.vector.reduce_sum(
                    out=yd, in_=windowed(bank, W, ROWS_PER_BANK),
                    axis=mybir.AxisListType.X,
                )
                nc.vector.reduce_sum(
                    out=qd, in_=windowed(sq[:, :], W, ROWS_PER_BANK),
                    axis=mybir.AxisListType.X,
                )

            # h-direction window sums, incremental: S[c, i, j] = sum_dh
            # Y[c, 4i+dh, j]. Window row i needs Y rows 4i..4i+7; after bank
            # n the rows up to 8n+7 exist, so rows {2n-1, 2n} (and row 0 for
            # n=0) become computable.
            sv = s_sb.rearrange("c (i j) -> c i j", j=OW)
            qv = ssq_sb.rearrange("c (i j) -> c i j", j=OW)

            def h_rows(i0, cnt):
                for src, dst in ((y_sb, sv), (ysq_sb, qv)):
                    ap = src[:, :, :]
                    win = bass.AP(
                        tensor=ap.tensor,
                        offset=ap.offset + i0 * STRIDE * OW,
                        ap=[list(ap.ap[0]), [STRIDE * OW, cnt], [1, OW], [OW, WIN]],
                    )
                    nc.vector.reduce_sum(
                        out=dst[:, i0:i0 + cnt, :], in_=win,
                        axis=mybir.AxisListType.X,
                    )

            # column-slab-major stream: all 32 contraction chunks for a
            # 4-bank column slab, then the next slab. Banks finish
            # progressively, so the windowed reduction overlaps the stream.
            for s in range(NSLAB):
                # one PSUM tile per bank so the post-stream reads never
                # create WAR hazards against later banks' matmuls
                c1bs = [
                    ps1.tile([C, 512], f32, tag="bank", bufs=NT, name=f"c1b{s}_{b}")
                    for b in range(SLAB_BANKS)
                ]
                for t in range(TPS):
                    at = affp.tile([128, KPT, SLAB_BANKS, 512], fmm)
                    nc.sync.dma_start(
                        out=at,
                        in_=aff[s, t].rearrange("j p (b n) -> p j b n", n=512),
                    )
                    for j in range(KPT):
                        kc = t * KPT + j
                        for b in range(SLAB_BANKS):
                            nc.tensor.matmul(
                                c1bs[b],
                                lhsT=gt_sb[:, 2 * kc:2 * kc + 2],
                                rhs=at[:, j, b, :],
                                start=(kc == 0),
                                stop=(kc == KC - 1),
                            )
                for b in range(SLAB_BANKS):
                    n = SLAB_BANKS * s + b
                    bank_postprocess(n, c1bs[b])
                    if n == 0:
                        h_rows(0, 1)
                    else:
                        h_rows(2 * n - 1, 2)

            nc.scalar.square(out=s2_sb, in_=s_sb)
            nc.vector.reduce_sum(out=out_sb[:, 0:1], in_=ssq_sb, axis=mybir.AxisListType.X)
            nc.vector.reduce_sum(out=out_sb[:, 1:2], in_=s2_sb, axis=mybir.AxisListType.X)
            nc.sync.dma_start(out=out[:, :], in_=out_sb)
    _split_multi_waits(nc)
    return nc


def _gt_host(grid_b):
    # grid_b: (64, 64, 2). g[c, p] = grid_b.reshape(4096, 2)[p, c]
    # gt layout: gt[p, 2*kc + c] = g[c, 128*kc + p]
    gt = np.ascontiguousarray(grid_b, dtype=np.float32).reshape(PIX, C)
    return np.ascontiguousarray(
        gt.reshape(KC, 128, C).transpose(1, 0, 2).reshape(128, 2 * KC)
    )


def run_cores(aff, grid, trace=False):
    """Compile (cached) and run the per-core bass kernel on cores 0..7.

    Returns the BassKernelResults from run_bass_kernel_spmd."""
    from concourse.bass_utils import run_bass_kernel_spmd

    if "nc" not in _CACHE:
        _CACHE["nc"] = _build_nc()
    nc = _CACHE["nc"]

    in_maps = []
    for b in range(B):
        # pack aff into DMA-tile order [slab, tile, j, p, n]: element
        # [(t*KPT + j)*128 + p, s*2048 + n] -> [s, t, j, p, n]
        a = np.ascontiguousarray(aff[b], dtype=np.float32)
        a = a.reshape(TPS, KPT, 128, NSLAB, SLAB_BANKS * 512).transpose(3, 0, 1, 2, 4)
        a = np.ascontiguousarray(a)
        in_maps.append(
            {
                "aff": np.ascontiguousarray(a),
                "gt": _gt_host(grid[b]),
            }
        )
    return run_bass_kernel_spmd(nc, in_maps, core_ids=list(range(B)), trace=trace)


def kernel(aff, grid):
    aff = np.asarray(aff, dtype=np.float32)
    grid = np.asarray(grid, dtype=np.float32)
    res = run_cores(aff, grid)
    total = 0.0
    for b in range(B):
        o = res.results[b]["out"].astype(np.float64)
        total += o[:, 0].sum() - o[:, 1].sum() / (WIN * WIN)
    total /= B * C * OH * OW * WIN * WIN
    return np.asarray(total, dtype=np.float32)


# revision 24
# speedup vs baseline: 1.1855x; 1.0346x over previous
"""Trainium2 Bass kernel for ConcentrationLoss.

Math (per batch element b, fully independent across b):
    g      = grid[b] viewed as (2, 4096)            # channels x pixels
    coord1 = g @ aff[b]                             # (2, 4096), the heavy op
    view coord1 as (2, 64, 64); extract 8x8 windows stride 4 -> 15x15 windows
    loss contribution = sum over windows w of [ sum_{p in w} x_p^2 - (sum_{p in w} x_p)^2 / 64 ]
    final = sum_b contribution_b / (8 * 2 * 225 * 64)

Sharding: batch b -> core b (8 cores). Each core streams its 64MB aff slice
through the TensorEngine (memory-bound), reduces the windowed variance on
device to per-channel partial sums, and the host combines the 8 partial
results into the scalar.

Device pipeline per core:
  - aff is host-packed into DMA-tile order so each 2MB transfer is fully
    contiguous with balanced 8KB descriptor runs, and streamed
    column-slab-major (all 32 contraction chunks of a slab, then the next
    slab) so PSUM banks complete progressively through the run.
  - Main matmul: out = lhsT.T @ rhs with lhsT = g^T chunks (128, 2) and
    rhs = aff tiles (128, 512), accumulated into one PSUM bank tile
    (2, 512) per 512-pixel column group. Operands are float32r: a
    single-pass matmul (plain fp32 LOW/HIGH-replays, streaming aff through
    the PE twice). fp32r rounds operands to ~12 mantissa bits; the
    end-to-end loss error stays ~1e-6 because the truncation noise
    averages out over the 230K-element mean.
  - When a bank finishes accumulating, its windowed reduction overlaps
    the remaining stream: square (ACT), then one 4-dim overlapping-AP
    tensor_reduce each for the w-direction window sums of x and x^2
    (DVE), then the newly-computable h-direction window rows.
  - The final (2, 2) output holds sum(SSq) and sum(S^2) per channel.
    Host: loss_b = sum_c [ sumSSq_c - sumS2_c / 64 ], all-reduced over
    the 8 cores on the host.
"""

import numpy as np

B = 8
C = 2
H = W = 64
PIX = H * W  # 4096, contraction dim
WIN = 8
STRIDE = 4
OH = OW = 15
KC = PIX // 128  # 32 contraction chunks of 128
NT = PIX // 512  # 8 psum-bank-wide output chunks (column slabs)
ROWS_PER_BANK = 512 // W  # 8 image rows per psum bank
SLAB_BANKS = 2   # psum banks (column groups) per slab
NSLAB = NT // SLAB_BANKS
KPT = 4          # contraction chunks per DMA tile (tile stays 2MB)
TPS = KC // KPT  # DMA tiles per slab
AFF_BUFS = 6
USE_F32R = True  # fp32r matmul: 1 cycle/col-pass (fp32 streams aff twice); ~2.8e-4 operand rounding

_CACHE = {}


def _split_multi_waits(nc, limit=1):
    """The walrus build in this toolchain rejects instructions carrying more
    than one sync wait (any template: CTRL, S3_LW, ...). Tile's scheduler
    freely emits multi-wait instructions. Post-process the scheduled BIR:
    hoist excess waits onto one-wait NoOps inserted immediately before the
    instruction on the same engine (sequencer waits are conjunctive and
    blocking, so semantics are identical)."""
    import concourse.mybir as mybir

    n_split = 0
    for f in nc.m.functions:
        for b in f.blocks:
            insts = b.instructions  # live view
            i = 0
            while i < len(insts):
                inst = insts[i]
                si = inst.sync_info
                if si is not None and len(si.on_wait) > limit:
                    waits = list(si.on_wait)
                    extra, keep = waits[:-limit], waits[-limit:]
                    for w in extra:
                        nop = mybir.InstNoOp(name=f"SWS-{n_split}")
                        n_split += 1
                        nop.engine = inst.engine
                        nop.sync_info = mybir.SyncInfo(on_wait=[w], on_update=[])
                        insts.insert(i, nop)
                        i += 1
                    inst.sync_info = mybir.SyncInfo(
                        on_wait=keep, on_update=si.on_update
                    )
                i += 1
    return n_split


def _build_nc():
    import concourse.bass as bass
    import concourse.mybir as mybir
    import concourse.tile as tile

    f32 = mybir.dt.float32
    fmm = mybir.dt.float32r if USE_F32R else f32
    nc = bass.Bass()
    # aff is pre-packed on the host into DMA-tile order [slab, tile, j, p, n]
    # (j = 8KB half-tile = one contraction chunk): every transfer is one
    # contiguous 2MB block whose descriptors split into balanced 8KB runs
    aff = nc.dram_tensor(
        "aff",
        [NSLAB, TPS, 2, 128, KPT // 2 * SLAB_BANKS * 512],
        fmm,
        kind="ExternalInput",
    )
    gt = nc.dram_tensor("gt", [128, 2 * KC], fmm, kind="ExternalInput")
    out = nc.dram_tensor("out", [C, 2], f32, kind="ExternalOutput")

    with tile.TileContext(nc) as tc:
        with (
            tc.tile_pool(name="consts", bufs=1) as consts,
            tc.tile_pool(name="small", bufs=1) as small,
            tc.tile_pool(name="sqp", bufs=2) as sqp,
            tc.tile_pool(name="affp", bufs=AFF_BUFS) as affp,
            tc.tile_pool(name="ps1", bufs=1, space="PSUM") as ps1,
        ):
            # consts go through SWDGE (gpsimd) so they never queue behind the
            # big aff stream on the HWDGE rings
            gt_sb = consts.tile([128, 2 * KC], fmm)
            nc.gpsimd.dma_start(out=gt_sb, in_=gt[:, :])

            y_sb = small.tile([C, H, OW], f32)      # w-windowsums of x
            ysq_sb = small.tile([C, H, OW], f32)    # w-windowsums of x^2
            s_sb = small.tile([C, OH * OW], f32)    # full window sums
            ssq_sb = small.tile([C, OH * OW], f32)  # full window sums of x^2
            s2_sb = small.tile([C, OH * OW], f32)   # S^2
            out_sb = small.tile([C, 2], f32)


            def windowed(ap, row_step, n_rows):
                """4-dim overlapping AP: [part, row, window j, dw] over a
                (C, n_rows*row_step) region; one tensor_reduce(X) gives the
                w-direction window sums in a single instruction."""
                return bass.AP(
                    tensor=ap.tensor,
                    offset=ap.offset,
                    ap=[list(ap.ap[0]), [row_step, n_rows], [STRIDE, OW], [1, WIN]],
                )

            def bank_postprocess(n, bank):
                """w-direction window sums for psum bank n; overlaps stream."""
                sq = sqp.tile([C, 512], f32, tag="sq")
                nc.scalar.square(out=sq, in_=bank)
                yd = y_sb[:, n * ROWS_PER_BANK:(n + 1) * ROWS_PER_BANK, :]
                qd = ysq_sb[:, n * ROWS_PER_BANK:(n + 1) * ROWS_PER_BANK, :]
                nc.vector.reduce_sum(
                    out=yd, in_=windowed(bank, W, ROWS_PER_BANK),
                    axis=mybir.AxisListType.X,
                )
                nc.vector.reduce_sum(
                    out=qd, in_=windowed(sq[:, :], W, ROWS_PER_BANK),
                    axis=mybir.AxisListType.X,
                )

            # h-direction window sums, incremental: S[c, i, j] = sum_dh
            # Y[c, 4i+dh, j]. Window row i needs Y rows 4i..4i+7; after bank
            # n the rows up to 8n+7 exist, so rows {2n-1, 2n} (and row 0 for
            # n=0) become computable.
            sv = s_sb.rearrange("c (i j) -> c i j", j=OW)
            qv = ssq_sb.rearrange("c (i j) -> c i j", j=OW)

            def h_rows(i0, cnt):
                for src, dst in ((y_sb, sv), (ysq_sb, qv)):
                    ap = src[:, :, :]
                    win = bass.AP(
                        tensor=ap.tensor,
                        offset=ap.offset + i0 * STRIDE * OW,
                        ap=[list(ap.ap[0]), [STRIDE * OW, cnt], [1, OW], [OW, WIN]],
                    )
                    nc.vector.reduce_sum(
                        out=dst[:, i0:i0 + cnt, :], in_=win,
                        axis=mybir.AxisListType.X,
                    )

            # column-slab-major stream: all 32 contraction chunks for a
            # 4-bank column slab, then the next slab. Banks finish
            # progressively, so the windowed reduction overlaps the stream.
            for s in range(NSLAB):
                # one PSUM tile per bank so the post-stream reads never
                # create WAR hazards against later banks' matmuls
                c1bs = [
                    ps1.tile([C, 512], f32, tag="bank", bufs=NT, name=f"c1b{s}_{b}")
                    for b in range(SLAB_BANKS)
                ]
                for t in range(TPS):
                    at = affp.tile([128, 2, KPT // 2, SLAB_BANKS, 512], fmm)
                    nc.sync.dma_start(
                        out=at,
                        in_=aff[s, t].rearrange(
                            "j p (q b n) -> p j q b n", q=KPT // 2, n=512
                        ),
                    )
                    for j in range(2):
                        for q in range(KPT // 2):
                            kc = t * KPT + j * (KPT // 2) + q
                            for b in range(SLAB_BANKS):
                                nc.tensor.matmul(
                                    c1bs[b],
                                    lhsT=gt_sb[:, 2 * kc:2 * kc + 2],
                                    rhs=at[:, j, q, b, :],
                                    start=(kc == 0),
                                    stop=(kc == KC - 1),
                                )
                for b in range(SLAB_BANKS):
                    n = SLAB_BANKS * s + b
                    bank_postprocess(n, c1bs[b])
                    if n == 0:
                        h_rows(0, 1)
                    else:
                        h_rows(2 * n - 1, 2)

            nc.scalar.square(out=s2_sb, in_=s_sb)
            nc.vector.reduce_sum(out=out_sb[:, 0:1], in_=ssq_sb, axis=mybir.AxisListType.X)
            nc.vector.reduce_sum(out=out_sb[:, 1:2], in_=s2_sb, axis=mybir.AxisListType.X)
            nc.sync.dma_start(out=out[:, :], in_=out_sb)
    _split_multi_waits(nc)
    return nc


def _gt_host(grid_b):
    # grid_b: (64, 64, 2). g[c, p] = grid_b.reshape(4096, 2)[p, c]
    # gt layout: gt[p, 2*kc + c] = g[c, 128*kc + p]
    gt = np.ascontiguousarray(grid_b, dtype=np.float32).reshape(PIX, C)
    return np.ascontiguousarray(
        gt.reshape(KC, 128, C).transpose(1, 0, 2).reshape(128, 2 * KC)
    )


def run_cores(aff, grid, trace=False):
    """Compile (cached) and run the per-core bass kernel on cores 0..7.

    Returns the BassKernelResults from run_bass_kernel_spmd."""
    from concourse.bass_utils import run_bass_kernel_spmd

    if "nc" not in _CACHE:
        _CACHE["nc"] = _build_nc()
    nc = _CACHE["nc"]

    in_maps = []
    for b in range(B):
        # pack aff into DMA-tile order [slab, tile, j(8KB half), p, (q cols)]:
        # element [(t*KPT + j*KPT//2 + q)*128 + p, s*slabw + n] -> [s,t,j,p,q,n]
        slabw = SLAB_BANKS * 512
        a = np.ascontiguousarray(aff[b], dtype=np.float32)
        a = a.reshape(TPS, 2, KPT // 2, 128, NSLAB, slabw).transpose(4, 0, 1, 3, 2, 5)
        a = np.ascontiguousarray(a).reshape(
            NSLAB, TPS, 2, 128, KPT // 2 * slabw
        )
        in_maps.append(
            {
                "aff": np.ascontiguousarray(a),
                "gt": _gt_host(grid[b]),
            }
        )
    return run_bass_kernel_spmd(nc, in_maps, core_ids=list(range(B)), trace=trace)


def kernel(aff, grid):
    aff = np.asarray(aff, dtype=np.float32)
    grid = np.asarray(grid, dtype=np.float32)
    res = run_cores(aff, grid)
    total = 0.0
    for b in range(B):
        o = res.results[b]["out"].astype(np.float64)
        total += o[:, 0].sum() - o[:, 1].sum() / (WIN * WIN)
    total /= B * C * OH * OW * WIN * WIN
    return np.asarray(total, dtype=np.float32)
